# revision 6
# baseline (speedup 1.0000x reference)
"""Trainium2 Bass kernel for nn_ClsCrossAttention (single-query CLS attention pooling).

Reference computation (per batch b, head h):
    tokens = features[b].reshape(C, H*W).T                  # [N=1024, C=768]
    K      = tokens @ W_k[h] + pos_embed                    # [N, 64]
    logits = K @ cls[h] / 8
    attn   = softmax(logits)
    out    = attn @ tokens                                  # [C]

Restructure (K is never materialized):
    logits[n, h] = tokens[n] . v_h + pos_bias[n, h]
        v_h      = W_k[h] @ cls[h] / 8          (host precomputed, [12, 768])
        pos_bias = pos_embed @ (cls/8)^T        (host precomputed, [1024, 12])
    Logits are ~+-0.02 so softmax needs no max subtraction. With d = exp(l)-1:
        out[h] = (colsum + d_h @ tokens) / (N + sum(d_h))
    colsum = sum_n tokens[n] computed exactly on the host in fp32.

Key PE structure: pooling needs tokens in n-major layout, i.e. PE transposes
of the c-major x tiles.  Each transpose matmul loads an x tile [128c, 128n]
as the stationary operand; a second 12-column matmul with the SAME
stationary (redundant LDWEIGHTS collapses to ~1ns) computes that tile's
n-major logit partial and ACCUMULATES it in PSUM across the 6 c-chunks —
the logits cost no extra weight loads and no vector-engine drain work.
Per batch the PE runs 48 transpose+logit pairs (~64ns each, LDW hidden)
+ 16 pooling matmuls (2 column-strip groups for array concurrency).

exp runs on the n-major [128, 96] logits read straight from PSUM (+pos
bias), so d = exp(l)-1 is directly the pooling stationary operand.

DMA: features fp32 -> bf16 cast during DMA (SWDGE), ~3.1 MB fp32 read per
batch; 8 batches per core saturate the ~360 GB/s per-core HBM read budget,
which is the roofline for this kernel.
"""

import sys

sys.path.insert(0, "/opt/trn_rl_repo")

import numpy as np
import ml_dtypes

import concourse.bass as bass
import concourse.mybir as mybir
from concourse import bacc
from concourse.tile import TileContext
from concourse.bass_utils import run_bass_kernel_spmd

BF16 = ml_dtypes.bfloat16

N_CORES = 8
B = 64
C = 768
N = 1024  # H*W = 32*32
NH = 12  # heads
DK = 64
BPC = B // N_CORES  # 8 batches per core
NCHUNK = C // 128  # 6 c-chunks
NTILE = N // 128  # 8 n-tiles
G = 2  # column-strip groups on the PE array for pooling
CHALF = C // G  # 384 output columns per group
# tokens_T layout: [c0..c383, ones, c384..c767, ones] -> 770 columns,
# each group's pooling rhs is a contiguous 385-column slice.
TOKW = C + G

_CACHE = {}


def _build_module():
    dt = mybir.dt
    nc = bacc.Bacc()

    feats = nc.dram_tensor("features", [BPC, C, N], dt.float32, kind="ExternalInput")
    colsum = nc.dram_tensor("colsum", [BPC, C], dt.float32, kind="ExternalInput")
    vT = nc.dram_tensor("vT", [128, NCHUNK, NH], dt.bfloat16, kind="ExternalInput")
    ident = nc.dram_tensor("ident", [128, 128], dt.bfloat16, kind="ExternalInput")
    posbT = nc.dram_tensor("posbT", [128, NTILE, NH], dt.float32, kind="ExternalInput")
    out = nc.dram_tensor("out", [BPC, NH, C], dt.float32, kind="ExternalOutput")

    with TileContext(nc) as tc:
        with (
            tc.tile_pool(name="consts", bufs=1) as consts,
            tc.tile_pool(name="xpool", bufs=3) as xpool,
            tc.tile_pool(name="tokpool", bufs=2) as tokpool,
            tc.tile_pool(name="sbmisc", bufs=2) as sbmisc,
            tc.tile_pool(name="tpsum", bufs=2, space="PSUM") as tpsum,
            tc.tile_pool(name="lpsum", bufs=2, space="PSUM") as lpsum,
            tc.tile_pool(name="ppsum", bufs=2, space="PSUM") as ppsum,
        ):
            vT_sb = consts.tile([128, NCHUNK, NH], dt.bfloat16)
            nc.sync.dma_start(out=vT_sb, in_=vT[:])
            id_sb = consts.tile([128, 128], dt.bfloat16)
            nc.sync.dma_start(out=id_sb, in_=ident[:])
            posb_sb = consts.tile([128, NTILE, NH], dt.float32)
            nc.sync.dma_start(out=posb_sb, in_=posbT[:])

            # colsum for all batches, broadcast to the 12 head rows of each
            # group's partition range, loaded once (emitted after batch 0's
            # feature load so it doesn't block startup on the SWDGE queue).
            cs_sb = consts.tile([44, BPC, CHALF], dt.float32)

            def emit_colsum():
                for g in range(G):
                    s = colsum[:, g * CHALF : (g + 1) * CHALF]  # [BPC, 384]
                    bcast = bass.AP(
                        tensor=s.tensor, offset=s.offset, ap=[[0, NH]] + s.ap
                    )
                    nc.gpsimd.dma_start(
                        out=cs_sb[32 * g : 32 * g + NH, :, :], in_=bcast
                    )

            state = {}  # per-batch tiles needed by the delayed (b-1) stages

            def emit_load(b):
                # fp32 -> bf16 cast during the DMA (SWDGE). Batch 0 loads per
                # chunk so the first matmul starts as early as possible;
                # later batches use one big DMA per half (SWDGE issue + drain
                # is ~1us per dma_start, so fewer is better once pipelined).
                x_sb = xpool.tile([128, NCHUNK, N], dt.bfloat16, name=f"x_{b}", tag="x")
                src = feats[b].rearrange("(k p) n -> p k n", p=128)
                if b == 0:
                    for k in range(NCHUNK):
                        nc.gpsimd.dma_start(
                            out=x_sb[:, k : k + 1, :], in_=src[:, k : k + 1, :]
                        )
                else:
                    half = NCHUNK // 2
                    for h in range(2):
                        ks = slice(h * half, (h + 1) * half)
                        nc.gpsimd.dma_start(out=x_sb[:, ks, :], in_=src[:, ks, :])
                return x_sb

            def emit_tok_alloc(b):
                tok_sb = tokpool.tile(
                    [128, NTILE, TOKW], dt.bfloat16, name=f"tok_{b}", tag="tok"
                )
                nc.gpsimd.memset(tok_sb[:, :, CHALF : CHALF + 1], 1.0)
                nc.gpsimd.memset(tok_sb[:, :, TOKW - 1 : TOKW], 1.0)
                lp = lpsum.tile([128, NTILE, NH], dt.float32, name=f"lp_{b}", tag="lp")
                return tok_sb, lp

            def emit_chunk(b, k, x_sb, tok_sb, lp):
                # chunk k's transpose column slot in tok
                col = 128 * k if k < 3 else CHALF + 1 + 128 * (k - 3)
                tp = tpsum.tile(
                    [128, NTILE, 128], dt.float32, name=f"tp_{b}_{k}", tag="tp"
                )
                for j in range(NTILE):
                    xt = x_sb[:, k, 128 * j : 128 * (j + 1)]
                    nc.tensor.matmul(
                        out=tp[:, j, :], lhsT=xt, rhs=id_sb[:], start=True, stop=True
                    )
                    # same stationary -> LDWEIGHTS is skipped; accumulates the
                    # n-major logit partial for this tile in PSUM across k.
                    nc.tensor.matmul(
                        out=lp[:, j, :],
                        lhsT=xt,
                        rhs=vT_sb[:, k, :],
                        start=(k == 0),
                        stop=(k == NCHUNK - 1),
                        skip_group_check=True,
                    )
                # drain the transposed chunk into tok; both engines take half
                # so the psum tile frees quickly (tpsum bufs=2)
                h = NTILE // 2
                nc.vector.tensor_copy(
                    tok_sb[:, 0:h, col : col + 128], tp[:, 0:h, :]
                )
                nc.scalar.copy(tok_sb[:, h:NTILE, col : col + 128], tp[:, h:NTILE, :])

            def emit_expd(b, lp):
                ltot = sbmisc.tile(
                    [128, NTILE, NH], dt.float32, name=f"lt_{b}", tag="lt"
                )
                nc.vector.tensor_add(ltot[:], lp[:], posb_sb[:])
                exp_sb = sbmisc.tile(
                    [128, NTILE, NH], dt.float32, name=f"exp_{b}", tag="exp"
                )
                nc.scalar.activation(
                    out=exp_sb[:],
                    in_=ltot[:],
                    func=mybir.ActivationFunctionType.Exp,
                )
                d_sb = sbmisc.tile([128, NTILE, NH], dt.bfloat16, name=f"d_{b}", tag="d")
                nc.vector.tensor_scalar_add(d_sb[:], exp_sb[:], -1.0)
                return d_sb

            def emit_pool(b, d_sb, tok_sb):
                pp = ppsum.tile([44, CHALF + 1], dt.float32, name=f"pp_{b}", tag="pp")
                # interleave the two column strips so the PE array runs both
                # concurrently (different col_grp strips).
                for j in range(NTILE):
                    for g in range(G):
                        lo = 32 * g
                        nc.tensor.matmul(
                            out=pp[lo : lo + NH, :],
                            lhsT=d_sb[:, j, :],
                            rhs=tok_sb[:, j, g * (CHALF + 1) : (g + 1) * (CHALF + 1)],
                            start=(j == 0),
                            stop=(j == NTILE - 1),
                        )
                for g in range(G):
                    lo = 32 * g
                    zt = sbmisc.tile([44, 1], dt.float32, name=f"z{g}_{b}", tag=f"z{g}")
                    nc.vector.tensor_scalar_add(
                        zt[lo : lo + NH, :],
                        pp[lo : lo + NH, CHALF : CHALF + 1],
                        float(N),
                    )
                    recip = sbmisc.tile(
                        [44, 1], dt.float32, name=f"r{g}_{b}", tag=f"r{g}"
                    )
                    nc.vector.reciprocal(
                        out=recip[lo : lo + NH, :], in_=zt[lo : lo + NH, :]
                    )
                    num = sbmisc.tile(
                        [44, CHALF], dt.float32, name=f"n{g}_{b}", tag=f"n{g}"
                    )
                    nc.vector.tensor_add(
                        num[lo : lo + NH, :],
                        pp[lo : lo + NH, 0:CHALF],
                        cs_sb[lo : lo + NH, b, :],
                    )
                    osb = sbmisc.tile(
                        [44, CHALF], dt.float32, name=f"o{g}_{b}", tag=f"o{g}"
                    )
                    nc.vector.tensor_scalar_mul(
                        osb[lo : lo + NH, :],
                        num[lo : lo + NH, :],
                        recip[lo : lo + NH, :],
                    )
                    nc.sync.dma_start(
                        out=out[b, :, g * CHALF : (g + 1) * CHALF],
                        in_=osb[lo : lo + NH, :],
                    )

            for b in range(BPC):
                x_sb = emit_load(b)
                if b == 0:
                    emit_colsum()
                tok_sb, lp = emit_tok_alloc(b)
                for k in range(3):
                    emit_chunk(b, k, x_sb, tok_sb, lp)
                if b > 0:
                    # previous batch's pooling between transpose bursts: if
                    # the second-half DMA lags, the PE has independent work
                    emit_pool(b - 1, state[b - 1]["d"], state[b - 1]["tok"])
                    del state[b - 1]
                for k in range(3, NCHUNK):
                    emit_chunk(b, k, x_sb, tok_sb, lp)
                d_sb = emit_expd(b, lp)
                state[b] = {"d": d_sb, "tok": tok_sb}

            bb = BPC - 1
            emit_pool(bb, state[bb]["d"], state[bb]["tok"])

    nc.compile()
    return nc


def _host_consts(cls, W_k, pos_embed):
    # v_h = W_k[h] @ cls[h] / 8;  lhsT layout [128, chunk, head]
    V = np.einsum("hcd,hd->hc", W_k.astype(np.float32), cls.astype(np.float32)) / 8.0
    vT = np.ascontiguousarray(
        V.T.reshape(NCHUNK, 128, NH).transpose(1, 0, 2)
    )  # vT[p, k, h] = V[h, 128k+p]
    ident = np.eye(128, dtype=np.float32)
    # pos bias in n-major: posb[n, h] = pos_embed[n] . cls[h] / 8
    posb = pos_embed[0, 0].astype(np.float32) @ (cls.astype(np.float32) / 8.0).T
    posbT = np.ascontiguousarray(posb.reshape(NTILE, 128, NH).transpose(1, 0, 2))
    return vT.astype(BF16), ident.astype(BF16), posbT.astype(np.float32)


def make_in_maps(features, cls, W_k, pos_embed):
    vT, ident, posbT = _host_consts(cls, W_k, pos_embed)
    x = features.reshape(B, C, N)
    colsum = x.sum(axis=2, dtype=np.float64).astype(np.float32)  # [B, C] exact
    in_maps = []
    for core in range(N_CORES):
        sl = slice(core * BPC, (core + 1) * BPC)
        in_maps.append(
            {
                "features": np.ascontiguousarray(x[sl]),
                "colsum": np.ascontiguousarray(colsum[sl]),
                "vT": vT,
                "ident": ident,
                "posbT": posbT,
            }
        )
    return in_maps


def kernel(features, cls, W_k, pos_embed):
    features = np.asarray(features, dtype=np.float32)
    cls = np.asarray(cls, dtype=np.float32)
    W_k = np.asarray(W_k, dtype=np.float32)
    pos_embed = np.asarray(pos_embed, dtype=np.float32)

    if "nc" not in _CACHE:
        _CACHE["nc"] = _build_module()
    nc = _CACHE["nc"]

    in_maps = make_in_maps(features, cls, W_k, pos_embed)
    res = run_bass_kernel_spmd(nc, in_maps, core_ids=list(range(N_CORES)))
    out = np.concatenate([r["out"] for r in res.results], axis=0)  # [64, 12, 768]
    return np.ascontiguousarray(out.reshape(B, NH * C)).astype(np.float32)
